# revision 1
# baseline (speedup 1.0000x reference)
"""MACE message-passing layer on 8 Trainium2 NeuronCores.

Strategy (graph-parallel / receiver-sharded):
  - Nodes are split into 8 contiguous ranges of 2048 (core 7: 1664 real).
  - Each edge is owned by the core owning its *receiver*; no collective is
    needed because every downstream consumer of the aggregated message
    (p2/p3 invariants, gate, readout) is local to the receiver node. The
    species-skip contribution to the scalar readout collapses to
    dot(node_feats[:, :, 0], (W_skip[s, 0] @ W_out)) and is evaluated on the
    owning core as well.
  - Within a core, nodes are processed in 16 windows of 128; each window's
    edges occupy a fixed budget of 18 edge tiles x 128 (padded; the one-hot
    scatter matrix zeroes out pads), so the instruction stream is identical
    on every core (SPMD).
  - Per edge tile: xs = node_feats[senders] is fetched with a descriptor
    DMA gather (bf16, 2KB rows), the radial MLP R runs on the PE, the
    depthwise tensor-product message is formed with DVE/GPSIMD elementwise
    ops, and segment-sum happens as a one-hot matmul accumulating into PSUM.
"""
import os
import sys

sys.path.insert(0, '/opt/trn_rl_repo')

import numpy as np
import ml_dtypes

import json

import concourse.bass as bass
import concourse.mybir as mybir
import concourse.tile as tile


def _split_waits(bir_bytes, max_waits=1):
    """This container's walrus build only encodes one sync-wait command per
    instruction; hoist excess on_wait entries onto preceding Drain carriers."""
    bir = json.loads(bir_bytes)
    for func in bir['functions']:
        for blk in func['blocks']:
            insts = blk.get('instructions')
            if not insts:
                continue
            out = []
            for inst in insts:
                si = inst.get('sync_info')
                waits = (si or {}).get('on_wait') or []
                if len(waits) > max_waits and inst.get('engine') != 'Unassigned':
                    excess, keep = waits[:-max_waits], waits[-max_waits:]
                    for i in range(0, len(excess), max_waits):
                        out.append({
                            'debug': inst.get('debug', 0),
                            'engine': inst['engine'],
                            'ins': [], 'outs': [],
                            'is_reset_sema': False,
                            'name': f"{inst['name']}ws{i}",
                            'opcode': 'Drain',
                            'sync_info': {'on_update': [],
                                          'on_wait': excess[i:i + max_waits]},
                        })
                    si['on_wait'] = keep
                out.append(inst)
            blk['instructions'] = out
    return json.dumps(bir).encode()


def _install_compile_patch():
    import concourse.bass_utils as bu
    import concourse.bass2jax as b2j
    if getattr(bu, "_mace_split_patch", False):
        return
    orig = bu.compile_bir_kernel

    def patched(bir_json, tmpdir, neff_name="file.neff"):
        return orig(_split_waits(bir_json), tmpdir, neff_name)

    bu.compile_bir_kernel = patched
    b2j.compile_bir_kernel = patched
    bu._mace_split_patch = True


_install_compile_patch()

BF16 = mybir.dt.bfloat16
F32 = mybir.dt.float32
I16 = mybir.dt.int16
AF = mybir.ActivationFunctionType
ALU = mybir.AluOpType
nbf16 = ml_dtypes.bfloat16

# ---- problem constants (hardcoded per contest rules) ----
N_NODES = 16000
N_EDGES = 256000
F = 64
LM = 16
NRAD = 8
EPS = 0.25
L_BLOCKS = [(0, 1), (1, 3), (4, 5), (9, 7)]  # (lm offset, size) per l

N_CORES = 8
NPC = 2048                 # node range per core (core 7: 1664 real + pad)
WPC = 16                   # windows of 128 nodes per core
TW = 18                    # edge tiles (x128) budget per window
TC = WPC * TW              # 288 tiles per core
EC = TC * 128              # 36864 edge slots per core
GT = 9                     # tiles per group (= per gather chunk)
NG = TC // GT              # 32 groups (2 per window)
GE = GT * 128              # 1152 edges per group

_SQ3 = float(np.sqrt(3.0))
_SQ15 = float(np.sqrt(15.0))
_S5H = float(np.sqrt(5.0) / 2.0)
_C358 = float(np.sqrt(35.0 / 8.0))
_C105 = float(np.sqrt(105.0))
_C218 = float(np.sqrt(21.0 / 8.0))
_C7H = float(np.sqrt(7.0) / 2.0)


def build_program():
    nc = bass.Bass()

    nf_d = nc.declare_dram_parameter("nf", [N_NODES, LM * F], BF16, isOutput=False)
    snd_d = nc.declare_dram_parameter("snd", [128, TC], mybir.dt.int32, isOutput=False)
    oh_d = nc.declare_dram_parameter("ohm", [128, TC * 128], BF16, isOutput=False)
    vec_d = nc.declare_dram_parameter("vec", [128, TC * 3], F32, isOutput=False)
    rad_d = nc.declare_dram_parameter("rad", [NRAD, EC], F32, isOutput=False)
    w1_d = nc.declare_dram_parameter("w1", [NRAD, F], F32, isOutput=False)
    b1_d = nc.declare_dram_parameter("b1", [F, 1], F32, isOutput=False)
    w2_d = nc.declare_dram_parameter("w2", [F, 256], BF16, isOutput=False)
    wq_d = nc.declare_dram_parameter("wq", [128, F], F32, isOutput=False)
    c2_d = nc.declare_dram_parameter("c2w", [128, WPC * F], F32, isOutput=False)
    c3_d = nc.declare_dram_parameter("c3w", [128, WPC * F], F32, isOutput=False)
    nf0_d = nc.declare_dram_parameter("nf0", [128, WPC * F], F32, isOutput=False)
    u_d = nc.declare_dram_parameter("usc", [128, WPC * F], F32, isOutput=False)
    out_d = nc.declare_dram_parameter("out", [128, WPC], F32, isOutput=True)
    h_d = nc.dram_tensor("h_dram", [F, EC], BF16)

    with tile.TileContext(nc) as tc:
        with (
            tc.tile_pool(name="const", bufs=1) as cpool,
            tc.tile_pool(name="hbuf", bufs=1) as hpool,
            tc.tile_pool(name="rad", bufs=2) as radpool,
            tc.tile_pool(name="hps", bufs=2, space="PSUM") as hps,
            tc.tile_pool(name="rps", bufs=2, space="PSUM") as rps,
            tc.tile_pool(name="aggps", bufs=2, space="PSUM") as aggps,
            tc.tile_pool(name="xs", bufs=2) as xspool,
            tc.tile_pool(name="grp", bufs=2) as gpool,
            tc.tile_pool(name="msg", bufs=2) as mpool,
            tc.tile_pool(name="node", bufs=2) as npool,
            tc.tile_pool(name="ysc", bufs=1) as ypool,
        ):
            # ---------- constants in ----------
            w1_t = cpool.tile([NRAD, F], F32)
            b1_t = cpool.tile([F, 1], F32)
            w2_t = cpool.tile([F, 256], BF16)
            wq_t = cpool.tile([128, F], F32)
            snd_t = cpool.tile([128, TC], mybir.dt.int32)
            vec_t = cpool.tile([128, TC * 3], F32)
            c2_t = cpool.tile([128, WPC * F], F32)
            c3_t = cpool.tile([128, WPC * F], F32)
            nf0_t = cpool.tile([128, WPC * F], F32)
            u_t = cpool.tile([128, WPC * F], F32)
            out_t = cpool.tile([128, WPC], F32)
            for t, d in [(w1_t, w1_d), (b1_t, b1_d), (w2_t, w2_d),
                         (wq_t, wq_d), (snd_t, snd_d),
                         (vec_t, vec_d), (c2_t, c2_d),
                         (c3_t, c3_d), (nf0_t, nf0_d), (u_t, u_d)]:
                nc.sync.dma_start(out=t[:], in_=d[:])

            # ---------- phase 1: h = silu(radial @ W1 + b1), transposed [F, EC] bf16 ----------
            # bounced through DRAM: a [64, EC] SBUF tile would reserve 72KB on
            # every partition.
            for c8 in range(EC // 2048):
                rad_ch = radpool.tile([NRAD, 2048], F32)
                nc.sync.dma_start(out=rad_ch[:], in_=rad_d[:, c8 * 2048:(c8 + 1) * 2048])
                h_ch = hpool.tile([F, 2048], BF16)
                for q in range(4):
                    hp = hps.tile([F, 512], F32)
                    nc.tensor.matmul(hp[:], lhsT=w1_t[:], rhs=rad_ch[:, q * 512:(q + 1) * 512],
                                     start=True, stop=True)
                    # silu(x+b) = (x+b) * sigmoid(x+b), split across ACT (2 ops)
                    # and DVE (bf16 2x multiply)
                    sg = hpool.tile([F, 512], BF16, tag="sg", bufs=2)
                    xb = hpool.tile([F, 512], BF16, tag="xb", bufs=2)
                    nc.scalar.activation(sg[:], hp[:], AF.Sigmoid, bias=b1_t[:], scale=1.0)
                    nc.scalar.activation(xb[:], hp[:], AF.Identity, bias=b1_t[:], scale=1.0)
                    nc.vector.tensor_tensor(h_ch[:, q * 512:(q + 1) * 512],
                                            xb[:], sg[:], ALU.mult)
                nc.sync.dma_start(out=h_d[:, c8 * 2048:(c8 + 1) * 2048], in_=h_ch[:])

            # ---------- phase 1b: spherical harmonics Y for all edge slots ----------
            # y_t[p, tile, m] (bf16), vec_t viewed [128, TC, 3]
            y_t = ypool.tile([128, TC * LM], BF16)
            y3 = y_t[:].rearrange("p (t m) -> p t m", t=TC)
            v3 = vec_t[:].rearrange("p (t j) -> p t j", t=TC)
            x, y, z = v3[:, :, 0], v3[:, :, 1], v3[:, :, 2]
            sc = [ypool.tile([128, TC], F32, name=f"ysc{i}") for i in range(8)]
            x2, y2, z2, s, xy, d_, t_, u_ = sc
            nc.vector.tensor_tensor(x2[:], x, x, ALU.mult)
            nc.vector.tensor_tensor(y2[:], y, y, ALU.mult)
            nc.vector.tensor_tensor(z2[:], z, z, ALU.mult)
            nc.vector.tensor_tensor(s[:], x2[:], y2[:], ALU.add)
            nc.vector.tensor_tensor(s[:], s[:], z2[:], ALU.add)
            nc.vector.tensor_scalar_add(s[:], s[:], 1e-12)
            nc.scalar.activation(s[:], s[:], AF.Sqrt)        # r
            nc.vector.reciprocal(s[:], s[:])                 # 1/r
            nx, ny, nz = x2, y2, z2  # reuse scratch for normalized coords
            nc.vector.tensor_tensor(nx[:], x, s[:], ALU.mult)
            nc.vector.tensor_tensor(ny[:], y, s[:], ALU.mult)
            nc.vector.tensor_tensor(nz[:], z, s[:], ALU.mult)
            # squares of normalized
            sx2, sy2, sz2 = s, xy, d_
            nc.vector.tensor_tensor(sx2[:], nx[:], nx[:], ALU.mult)
            nc.vector.tensor_tensor(sy2[:], ny[:], ny[:], ALU.mult)
            nc.vector.tensor_tensor(sz2[:], nz[:], nz[:], ALU.mult)
            nc.vector.memset(y3[:, :, 0], 1.0)
            nc.vector.tensor_scalar_mul(y3[:, :, 1], ny[:], _SQ3)
            nc.vector.tensor_scalar_mul(y3[:, :, 2], nz[:], _SQ3)
            nc.vector.tensor_scalar_mul(y3[:, :, 3], nx[:], _SQ3)
            nc.vector.scalar_tensor_tensor(y3[:, :, 4], nx[:], _SQ15, ny[:], ALU.mult, ALU.mult)
            nc.vector.scalar_tensor_tensor(y3[:, :, 5], ny[:], _SQ15, nz[:], ALU.mult, ALU.mult)
            nc.vector.tensor_scalar(y3[:, :, 6], sz2[:], 3.0 * _S5H, -_S5H, ALU.mult, ALU.add)
            nc.vector.scalar_tensor_tensor(y3[:, :, 7], nx[:], _SQ15, nz[:], ALU.mult, ALU.mult)
            nc.vector.tensor_tensor(t_[:], sx2[:], sy2[:], ALU.subtract)   # x2-y2
            nc.vector.tensor_scalar_mul(y3[:, :, 8], t_[:], _SQ15 / 2.0)
            nc.vector.scalar_tensor_tensor(y3[:, :, 14], t_[:], _C105 / 2.0, nz[:], ALU.mult, ALU.mult)
            # lm9 = c358*ny*(3x2-y2); lm15 = c358*nx*(x2-3y2)
            nc.vector.tensor_scalar(u_[:], sx2[:], 3.0, None, ALU.mult)
            nc.vector.tensor_tensor(u_[:], u_[:], sy2[:], ALU.subtract)
            nc.vector.scalar_tensor_tensor(y3[:, :, 9], u_[:], _C358, ny[:], ALU.mult, ALU.mult)
            nc.vector.tensor_scalar(u_[:], sy2[:], 3.0, None, ALU.mult)
            nc.vector.tensor_tensor(u_[:], sx2[:], u_[:], ALU.subtract)
            nc.vector.scalar_tensor_tensor(y3[:, :, 15], u_[:], _C358, nx[:], ALU.mult, ALU.mult)
            # lm10 = c105*nx*ny*nz
            nc.vector.tensor_tensor(u_[:], nx[:], ny[:], ALU.mult)
            nc.vector.scalar_tensor_tensor(y3[:, :, 10], u_[:], _C105, nz[:], ALU.mult, ALU.mult)
            # lm11/13: c218*{ny,nx}*(5z2-1)
            nc.vector.tensor_scalar(u_[:], sz2[:], 5.0, -1.0, ALU.mult, ALU.add)
            nc.vector.scalar_tensor_tensor(y3[:, :, 11], u_[:], _C218, ny[:], ALU.mult, ALU.mult)
            nc.vector.scalar_tensor_tensor(y3[:, :, 13], u_[:], _C218, nx[:], ALU.mult, ALU.mult)
            # lm12 = c7h*nz*(5z2-3)
            nc.vector.tensor_scalar(u_[:], sz2[:], 5.0, -3.0, ALU.mult, ALU.add)
            nc.vector.scalar_tensor_tensor(y3[:, :, 12], u_[:], _C7H, nz[:], ALU.mult, ALU.mult)

            # ---------- phase 2: message passing ----------
            for w in range(WPC):
                agg = aggps.tile([128, LM * F], F32, space="PSUM")
                for gg in range(2):
                    g = w * 2 + gg
                    xs = xspool.tile([128, GT, LM * F], BF16)
                    for t in range(GT):
                        # HW DGE supports one gather index per partition
                        nc.gpsimd.indirect_dma_start(
                            out=xs[:, t, :], out_offset=None, in_=nf_d[:],
                            in_offset=bass.IndirectOffsetOnAxis(
                                ap=snd_t[:, g * GT + t:g * GT + t + 1], axis=0))
                    # one-hot scatter matrix [e_part, window_col] (host-built)
                    oh = gpool.tile([128, GT * 128], BF16)
                    nc.sync.dma_start(out=oh[:], in_=oh_d[:, g * GT * 128:(g + 1) * GT * 128])
                    # R = h @ W2, per tile, f32 psum -> bf16 sbuf (l-major, f-minor)
                    h_g = gpool.tile([F, GE], BF16, tag="h_g")
                    nc.sync.dma_start(out=h_g[:], in_=h_d[:, g * GE:(g + 1) * GE])
                    r_sb = gpool.tile([128, GT * 256], BF16)
                    for t in range(GT):
                        rp = rps.tile([128, 256], F32, space="PSUM")
                        nc.tensor.matmul(rp[:], lhsT=h_g[:, t * 128:(t + 1) * 128], rhs=w2_t[:],
                                         start=True, stop=True)
                        nc.scalar.activation(r_sb[:, t * 256:(t + 1) * 256], rp[:], AF.Copy)
                    r3 = r_sb[:].rearrange("p (t x) -> p t x", t=GT)
                    # B = R * xs0 (broadcast over l)
                    b_sb = gpool.tile([128, GT * 256], BF16)
                    nc.vector.tensor_tensor(
                        b_sb[:].rearrange("p (t l f) -> p t l f", t=GT, l=4),
                        r_sb[:].rearrange("p (t l f) -> p t l f", t=GT, l=4),
                        xs[:, :, 0:F].unsqueeze(2).to_broadcast([128, GT, 4, F]),
                        ALU.mult)
                    b3 = b_sb[:].rearrange("p (t x) -> p t x", t=GT)
                    msg = mpool.tile([128, GT, LM * F], BF16)
                    tmp = mpool.tile([128, GT, LM * F], BF16, bufs=1)
                    yg = y_t[:].rearrange("p (t m) -> p t m", t=TC)[:, g * GT:(g + 1) * GT, :]
                    for li, (off, sz) in enumerate(L_BLOCKS):
                        nc.vector.tensor_tensor(
                            msg[:, :, off * F:(off + sz) * F].rearrange("p t (m f) -> p t m f", m=sz),
                            xs[:, :, off * F:(off + sz) * F].rearrange("p t (m f) -> p t m f", m=sz),
                            r3[:, :, li * F:(li + 1) * F].unsqueeze(2).to_broadcast([128, GT, sz, F]),
                            ALU.mult)
                        nc.vector.tensor_tensor(
                            tmp[:, :, off * F:(off + sz) * F].rearrange("p t (m f) -> p t m f", m=sz),
                            b3[:, :, li * F:(li + 1) * F].unsqueeze(2).to_broadcast([128, GT, sz, F]),
                            yg[:, :, off:off + sz].unsqueeze(3).to_broadcast([128, GT, sz, F]),
                            ALU.mult)
                    # scatter: agg[w_node, :] += onehot^T @ (msg and tmp) —
                    # PSUM accumulation performs the msg1+msg2 add for free.
                    for t in range(GT):
                        first = (gg == 0 and t == 0)
                        last = (gg == 1 and t == GT - 1)
                        for half in range(2):
                            nc.tensor.matmul(
                                agg[:, half * 512:(half + 1) * 512],
                                lhsT=oh[:, t * 128:(t + 1) * 128],
                                rhs=msg[:, t, half * 512:(half + 1) * 512],
                                start=first, stop=False, skip_group_check=True)
                            nc.tensor.matmul(
                                agg[:, half * 512:(half + 1) * 512],
                                lhsT=oh[:, t * 128:(t + 1) * 128],
                                rhs=tmp[:, t, half * 512:(half + 1) * 512],
                                start=False, stop=last, skip_group_check=True)
                # ---------- node phase for window w ----------
                sq = npool.tile([128, LM * F], F32, tag="sq")
                nc.scalar.activation(sq[:], agg[:], AF.Square)
                sq3 = sq[:].rearrange("p (m f) -> p m f", m=LM)
                s8 = npool.tile([128, 8 * F], F32, tag="s8")
                s83 = s8[:].rearrange("p (m f) -> p m f", m=8)
                nc.vector.tensor_tensor(s83, sq3[:, 0:8, :], sq3[:, 8:16, :], ALU.add)
                s4 = npool.tile([128, 4 * F], F32, tag="s4")
                s43 = s4[:].rearrange("p (m f) -> p m f", m=4)
                nc.vector.tensor_tensor(s43, s83[:, 0:4, :], s83[:, 4:8, :], ALU.add)
                p2 = npool.tile([128, F], F32, tag="p2")
                nc.vector.tensor_tensor(s4[:, 0:F], s4[:, 0:F], s4[:, F:2 * F], ALU.add)
                nc.vector.tensor_tensor(s4[:, 2 * F:3 * F], s4[:, 2 * F:3 * F], s4[:, 3 * F:4 * F], ALU.add)
                nc.vector.tensor_tensor(p2[:], s4[:, 0:F], s4[:, 2 * F:3 * F], ALU.add)
                a0 = npool.tile([128, F], F32, tag="a0")
                nc.vector.tensor_copy(a0[:], agg[:, 0:F])
                t1 = npool.tile([128, F], F32, tag="t1")
                nc.vector.tensor_tensor(t1[:], p2[:], a0[:], ALU.mult)
                nc.vector.tensor_tensor(t1[:], t1[:], c3_t[:, w * F:(w + 1) * F], ALU.mult)
                t3 = npool.tile([128, F], F32, tag="t3")
                nc.vector.tensor_tensor(t3[:], p2[:], c2_t[:, w * F:(w + 1) * F], ALU.mult)
                gate = npool.tile([128, F], F32, tag="gate")
                nc.vector.scalar_tensor_tensor(gate[:], t3[:], 1.0, t1[:],
                                               ALU.add, ALU.add)
                q = npool.tile([128, F], F32, tag="q")
                nc.vector.tensor_tensor(q[:], a0[:], gate[:], ALU.mult)
                scr = npool.tile([128, F], F32, tag="scr")
                scr2 = npool.tile([128, F], F32, tag="scr2")
                nc.vector.tensor_tensor(scr[:], q[:], wq_t[:], ALU.mult)
                nc.vector.tensor_tensor(scr2[:], nf0_t[:, w * F:(w + 1) * F],
                                        u_t[:, w * F:(w + 1) * F], ALU.mult)
                nc.vector.tensor_tensor(scr[:], scr[:], scr2[:], ALU.add)
                nc.vector.tensor_reduce(out_t[:, w:w + 1], scr[:],
                                        mybir.AxisListType.X, ALU.add)

            nc.sync.dma_start(out=out_d[:], in_=out_t[:])
    return nc


def host_prep(inputs):
    """Build the 8 per-core input maps + metadata for output assembly."""
    vectors = np.asarray(inputs["vectors"], np.float32)
    node_feats = np.asarray(inputs["node_feats"], np.float32)
    radial = np.asarray(inputs["radial_embedding"], np.float32)
    node_specie = np.asarray(inputs["node_specie"]).astype(np.int64)
    senders = np.asarray(inputs["senders"]).astype(np.int64)
    receivers = np.asarray(inputs["receivers"]).astype(np.int64)
    W_rad1 = np.asarray(inputs["W_rad1"], np.float32)
    b_rad1 = np.asarray(inputs["b_rad1"], np.float32)
    W_rad2 = np.asarray(inputs["W_rad2"], np.float32)
    W_skip = np.asarray(inputs["W_skip"], np.float32)
    c2 = np.asarray(inputs["c2"], np.float32)
    c3 = np.asarray(inputs["c3"], np.float32)
    W_out = np.asarray(inputs["W_out"], np.float32)

    # shared tensors
    nf_g = np.ascontiguousarray(
        node_feats.transpose(0, 2, 1).reshape(N_NODES, LM * F)).astype(nbf16)
    w2lf = np.ascontiguousarray(
        W_rad2.reshape(F, F, 4).transpose(0, 2, 1).reshape(F, 4 * F)).astype(nbf16)
    wq = np.tile((EPS * W_out[:, 0])[None, :], (128, 1)).astype(np.float32)
    u_sp = np.einsum('sfg,g->sf', W_skip[:, 0], W_out[:, 0])  # [10, F]
    U = u_sp[node_specie]                                     # [N, F]
    c2n = c2[node_specie] * (EPS ** 2)
    c3n = c3[node_specie] * (EPS ** 3)
    nf0 = node_feats[:, :, 0]                                 # [N, F]

    def node_layout(arr):  # [NPC_real, F] padded -> [128, WPC*F]
        out = np.zeros((WPC, 128, F), np.float32)
        out.reshape(-1, F)[:arr.shape[0]] = arr
        return np.ascontiguousarray(out.transpose(1, 0, 2).reshape(128, WPC * F))

    core_of = receivers // NPC
    win_of = (receivers % NPC) // 128

    in_maps = []
    for c in range(N_CORES):
        snd_c = np.zeros(EC, np.int64)
        rcv_c = np.full(EC, 192.0, np.float32)
        vec_c = np.zeros((EC, 3), np.float32)
        rad_c = np.zeros((EC, NRAD), np.float32)
        for w in range(WPC):
            e_idx = np.nonzero((core_of == c) & (win_of == w))[0]
            ne = e_idx.size
            assert ne <= TW * 128, f"window overflow: core {c} win {w}: {ne}"
            base = w * TW * 128
            snd_c[base:base + ne] = senders[e_idx]
            rcv_c[base:base + ne] = (receivers[e_idx] - (c * NPC + w * 128)).astype(np.float32)
            vec_c[base:base + ne] = vectors[e_idx]
            rad_c[base:base + ne] = radial[e_idx]
        n_lo = c * NPC
        n_hi = min(N_NODES, n_lo + NPC)
        # host-built one-hot scatter matrices: oh[p, t*128 + w] =
        # (recv_local[t*128+p] == w), pads (sentinel 192) never match.
        oh = (rcv_c.reshape(TC, 128).T[:, :, None]
              == np.arange(128, dtype=np.float32)[None, None, :])
        in_maps.append({
            "nf": nf_g,
            "snd": np.ascontiguousarray(
                snd_c.reshape(TC, 128).T.astype(np.int32)),
            "ohm": np.ascontiguousarray(oh.reshape(128, TC * 128)).astype(nbf16),
            "vec": np.ascontiguousarray(
                vec_c.reshape(TC, 128, 3).transpose(1, 0, 2).reshape(128, TC * 3)),
            "rad": np.ascontiguousarray(rad_c.T),
            "w1": W_rad1,
            "b1": b_rad1[:, None].copy(),
            "w2": w2lf,
            "wq": wq,
            "c2w": node_layout(c2n[n_lo:n_hi]),
            "c3w": node_layout(c3n[n_lo:n_hi]),
            "nf0": node_layout(nf0[n_lo:n_hi]),
            "usc": node_layout(U[n_lo:n_hi]),
        })
    return in_maps


def assemble_output(results):
    """results: list of 8 dicts with 'out' [128, WPC] -> [N_NODES, 1] f32."""
    full = np.zeros((N_CORES * NPC,), np.float32)
    for c in range(N_CORES):
        o = np.asarray(results[c]["out"], np.float32)  # [128, WPC]
        full[c * NPC:(c + 1) * NPC] = o.T.reshape(-1)
    return full[:N_NODES, None].copy()


_CACHED_NC = None
LAST_EXEC_NS = None
LAST_RESULTS = None


def kernel(**inputs):
    global _CACHED_NC, LAST_EXEC_NS, LAST_RESULTS
    from concourse.bass_utils import run_bass_kernel_spmd
    in_maps = host_prep(inputs)
    if _CACHED_NC is None:
        _CACHED_NC = build_program()
    trace = bool(int(os.environ.get("MACE_TRACE", "0")))
    kwargs = {}
    if trace:
        kwargs.update(trace=True, trace_cores=[0], tmpdir="/root/problem/trace_out")
        os.makedirs("/root/problem/trace_out", exist_ok=True)
    res = run_bass_kernel_spmd(_CACHED_NC, in_maps, list(range(N_CORES)), **kwargs)
    LAST_EXEC_NS = res.exec_time_ns
    LAST_RESULTS = res
    return assemble_output(res.results)

